# revision 15
# baseline (speedup 1.0000x reference)
# Trainium2 Bass kernel for a binarized 2-block MLP (BNN):
#   h1 = sign(BN1(x @ sign(w1).T + b1)); h2 = sign(BN2(h1 @ sign(w2).T + b2))
#   out = log_softmax(h2 @ sign(w5).T + b5)
#
# Strategy: pure data parallel over 8 NeuronCores (batch sharded, weights
# replicated). Host-side prep:
#   * x is split into fp16 hi/lo parts (x == xh + xl/2048 to ~2^-22 in fp32).
#     Both stream through the PE at full (1 col/cycle) rate vs fp32's 4
#     passes; with +-1 binary weights every product is exact in the PE's
#     FP22 pipe, so the result matches a true fp32 matmul to accumulation
#     order. hi/lo are interleaved per chunk in ONE dram tensor so each
#     512-row chunk is a single 2MB DMA with 16KB-contiguous runs per
#     partition (HBM line rate).
#   * BN is folded into per-feature scale/shift applied inside the Sign
#     activation (ACT computes func(scale*in + bias) for free).
#   * b5 is folded in on the vector engine: sum(exp(mm+b5)) via a fused
#     multiply-reduce against exp(b5), and the final subtract adds b5 in the
#     same fused op.
#
# Schedule (v2): chunk-granular (512 rows) software pipeline. The kernel is
# HBM-bound: per core 32MB of x must stream at ~358GB/s (~90us) while the PE
# only has ~70us of work, so the PE necessarily idles ~1.3us per chunk. Left
# alone, those idle gaps re-throttle the PE's HAM clock gate to 1.2GHz
# (observed: 58us of K=4/8 in the v1 trace), making the PE the bottleneck.
# Fixes:
#   * fc5's PSUM (and all inter-stage PSUM) double-buffered -- v1 had
#     bufs=1 on ps5, forcing each chunk's fc5 to wait for the previous
#     chunk's full softmax chain (~2.5us PE stall per pair).
#   * "keep-warm" filler matmuls (fp32 ident @ ident, one long accumulation
#     group into a junk PSUM bank) pad the PE's DMA-wait gaps so the HAM
#     activity monitor never sees an idle window: warmup burst at t=0, then
#     a few per chunk.
#   * x chunks stream on the sync (SP) HWDGE ring with nothing else on it;
#     weights/consts go on the scalar (ACT) HWDGE ring.
#   * output stores go via gpsimd SWDGE (keeps SP free); the final store
#     uses the by-then-idle sync ring for lower latency.
# fc5 output lands batch-on-partitions (y2 block as the stationary operand)
# so log_softmax reduces along the free dim; the result is PE-transposed so
# the store DMA has 512B-contiguous runs; host reassembles.

import os
import sys

import numpy as np

for _p in ("/opt/trn_rl_repo", "/root/.axon_site/_ro/trn_rl_repo"):
    if os.path.isdir(_p) and _p not in sys.path:
        sys.path.insert(0, _p)

import concourse.bass as bass
import concourse.mybir as mybir
import concourse.tile as tile
from concourse import bacc
from concourse.masks import make_identity

N_CORES = 8
B, D, H1, H2, O = 65536, 1024, 50, 20, 10
BPC = B // N_CORES  # batch rows per core
CH = 512            # batch chunk (one PSUM bank of fp32)
KS = D // 128       # contraction slices
EPS = 1e-4
LO = 2048.0         # lo-part scale (2**11)

NBUF = 6            # x chunk buffers in flight (absorbs PE lag jitter so the
                    # DMA stream never stalls on buffer recycling)
WARMUP_FILL = 20    # filler matmuls before chunk 0 lands (~6.5us of PE busy)
# per-iteration filler counts: at the real 2.0GHz P0 clock the PE is
# already near-balanced with the ~5.3-5.7us chunk DMA period, so the
# pipeline needs no steady-state fillers (and anchored fillers can stall
# the in-order PE queue if scheduled before their chunk lands).
FILL_AT = {}
FILL_DEFAULT = 0

F16 = mybir.dt.float16
F32 = mybir.dt.float32
AF = mybir.ActivationFunctionType
AX = mybir.AxisListType
OP = mybir.AluOpType


def build_bass(bpc: int = BPC) -> bass.Bass:
    nch = bpc // CH
    npair = nch // 2
    nc = bacc.Bacc("TRN2", target_bir_lowering=False)

    # All activations used here (Sign, Exp, Ln) live together in the
    # "natural_log_exp_and_others" ACT table set, but the default chooser
    # first-matches Exp->exp_and_others and Ln->natural_log, reloading
    # tables twice per chunk (~2.7us each). Restrict the chooser to the
    # combined set (other entries emptied so indices stay aligned with
    # act_info.json) -> exactly one table load for the whole kernel.
    def _act_table_loads_combined_set_only(self=nc):
        import bass_rust as _br

        from concourse.hw_specs import get_activation_tables

        has_act = any(
            isinstance(i, mybir.InstActivation)
            for blk in self.main_func.blocks
            for i in blk.instructions
        )
        if not has_act:
            return
        tabs = get_activation_tables(self.m.arch)
        tables = [
            (name, fns if name == "natural_log_exp_and_others" else set())
            for name, fns in tabs.items()
        ]
        _br.insert_act_table_loads(self, tables)

    nc.insert_act_table_loads = _act_table_loads_combined_set_only

    # x arrives pre-swizzled with hi/lo interleaved per chunk:
    #   x2[p, c, s, k, n] = (xh if s==0 else xl).T[k*128+p, c*CH+n]
    # so each chunk's per-partition DMA slice is one contiguous 16KB run.
    x2 = nc.declare_dram_parameter("x2", [128, nch, 2, KS, CH], F16, isOutput=False)
    w1h = nc.declare_dram_parameter("w1h", [D, H1], F16, isOutput=False)
    w1l = nc.declare_dram_parameter("w1l", [D, H1], F16, isOutput=False)
    w2t = nc.declare_dram_parameter("w2t", [H1, H2], F16, isOutput=False)
    w5t = nc.declare_dram_parameter("w5t", [H2, O], F16, isOutput=False)
    cs1 = nc.declare_dram_parameter("cs1", [H1, 2], F32, isOutput=False)
    cs2 = nc.declare_dram_parameter("cs2", [H2, 2], F32, isOutput=False)
    b5r = nc.declare_dram_parameter("b5r", [128, 4 * O], F32, isOutput=False)
    eb5r = nc.declare_dram_parameter("eb5r", [128, 4 * O], F32, isOutput=False)
    # Output, transposed pair blocks:
    #   y[c*4*O + j*O + o, pr*128 + p] = out[pr*1024 + c*512 + j*128 + p, o]
    y = nc.declare_dram_parameter("y", [8 * O, npair * 128], F32, isOutput=True)

    with tile.TileContext(nc) as tc:
        from contextlib import ExitStack

        with ExitStack() as ctx:
            singles = ctx.enter_context(tc.tile_pool(name="singles", bufs=1))
            xpool = ctx.enter_context(tc.tile_pool(name="xpool", bufs=NBUF))
            mids = ctx.enter_context(tc.tile_pool(name="mids", bufs=3))
            outs = ctx.enter_context(tc.tile_pool(name="outs", bufs=3))
            p1pool = ctx.enter_context(tc.tile_pool(name="p1", bufs=2, space="PSUM"))
            p2pool = ctx.enter_context(tc.tile_pool(name="p2", bufs=2, space="PSUM"))
            p5pool = ctx.enter_context(tc.tile_pool(name="p5", bufs=2, space="PSUM"))
            pTpool = ctx.enter_context(tc.tile_pool(name="pT", bufs=1, space="PSUM"))
            pFpool = ctx.enter_context(tc.tile_pool(name="pF", bufs=1, space="PSUM"))

            # Identity first: the gpsimd engine builds it in ~1us so the
            # warmup fillers can start right after the preamble.
            ident = singles.tile([128, 128], F32)
            make_identity(nc, ident)
            identb = singles.tile([128, 128], F32)
            make_identity(nc, identb)

            # Each x chunk is TWO 1MB DMAs: hi on the sync (SP) HWDGE ring,
            # lo on the gpsimd SWDGE queue. Two reasons:
            #  * The Tile scheduler emits per-engine order from a timed
            #    simulation whose DMA model underestimates HBM rate; with
            #    one queue it concludes the PE must wait for each chunk and
            #    emits a serialized fc1->sign->fc2 chain. Two parallel
            #    queues double the modeled stream rate so the emitted order
            #    keeps the software pipeline.
            #  * Both queues carry the SAME chunk, so the 16 SDMA engines'
            #    packet round-robin still completes chunks in FIFO order at
            #    the full HBM rate. (Alternating whole chunks across queues
            #    instead halves each chunk's rate and delays its completion
            #    semaphore by a full chunk period.)
            # SWDGE descriptor generation runs ~NBUF chunks ahead on the
            # otherwise-idle gpsimd Q7, and the ACT engine is left with
            # activations + store descriptor-gen only, so sign1(c) is always
            # delivered while fc1(c+1) streams.
            xts = []

            def issue_x(c):
                xh_t = xpool.tile([128, KS, CH], F16, tag="xh", name="xh_t")
                nc.sync.dma_start(out=xh_t, in_=x2[:, c, 0])
                xl_t = xpool.tile([128, KS, CH], F16, tag="xl", name="xl_t")
                nc.gpsimd.dma_start(out=xl_t, in_=x2[:, c, 1])
                xts.append((xh_t, xl_t))

            issue_x(0)
            issue_x(1)
            # --- constants / weights: sync ring, slotted behind the first
            # two hi chunks -- they land ~12us in (fc1(0) needs w1 at ~15us)
            # without delaying the first chunk's arrival. ---
            w1h_sb = singles.tile([128, KS, H1], F16)
            nc.sync.dma_start(out=w1h_sb, in_=w1h.rearrange("(k p) m -> p k m", p=128))
            w1l_sb = singles.tile([128, KS, H1], F16)
            nc.sync.dma_start(out=w1l_sb, in_=w1l.rearrange("(k p) m -> p k m", p=128))
            w2_sb = singles.tile([H1, H2], F16)
            nc.sync.dma_start(out=w2_sb, in_=w2t[:, :])
            w5_sb = singles.tile([H2, O], F16)
            nc.sync.dma_start(out=w5_sb, in_=w5t[:, :])
            cs1_sb = singles.tile([H1, 2], F32)
            nc.sync.dma_start(out=cs1_sb, in_=cs1[:, :])
            cs2_sb = singles.tile([H2, 2], F32)
            nc.sync.dma_start(out=cs2_sb, in_=cs2[:, :])
            b5r_sb = singles.tile([128, 4 * O], F32)
            nc.sync.dma_start(out=b5r_sb, in_=b5r[:, :])
            eb5r_sb = singles.tile([128, 4 * O], F32)
            nc.sync.dma_start(out=eb5r_sb, in_=eb5r[:, :])
            for c in range(2, min(NBUF, nch)):
                issue_x(c)

            def fillers(n, xt=None):
                # Keep-warm matmuls: pad PE idle so the HAM activity monitor
                # never re-throttles the clock. One accumulation group -> no
                # per-instruction WAW semaphore stalls; output never read.
                # Warmup form (xt=None): fp32 ident@ident, dependency-free
                # so it runs during the pre-chunk-0 window. In-loop form:
                # reads chunk c's x tile, anchoring it to iteration c (a
                # dependency-free filler would be hoisted to t=0 by the
                # scheduler, bunching all fillers at the start).
                if n <= 0:
                    return
                fp = pFpool.tile([128, CH], F32, tag="f", name="fill")
                if xt is None:
                    for i in range(n):
                        nc.tensor.matmul(fp[:, 0:128],
                                         lhsT=(ident if i % 2 == 0 else identb),
                                         rhs=ident,
                                         start=(i == 0), stop=(i == n - 1))
                else:
                    for i in range(n):
                        nc.tensor.matmul(fp, lhsT=xt[:, 0, 0:128],
                                         rhs=xt[:, 0, :],
                                         start=(i == 0), stop=(i == n - 1))

            def fc1(c):
                xh_t, xl_t = xts[c]
                ps1 = p1pool.tile([H1, CH], F32, tag="ps1", name="ps1")
                # all hi k-slices first: the lo half rides the busier ACT
                # ring and may land ~1us later; its wait sits later in the
                # PE stream so it overlaps the hi matmuls.
                for k in range(KS):
                    nc.tensor.matmul(ps1, lhsT=w1h_sb[:, k, :], rhs=xh_t[:, k, :],
                                     start=(k == 0), stop=False)
                for k in range(KS):
                    nc.tensor.matmul(ps1, lhsT=w1l_sb[:, k, :], rhs=xl_t[:, k, :],
                                     start=False, stop=(k == KS - 1))
                return ps1

            def stage_a(ps1):
                """sign1 -> fc2 -> sign2 for one chunk; returns y2."""
                y1 = mids.tile([H1, CH], F16, tag="y1", name="y1")
                nc.scalar.activation(y1, ps1, AF.Sign,
                                     bias=cs1_sb[:, 1:2], scale=cs1_sb[:, 0:1])
                ps2 = p2pool.tile([H2, CH], F32, tag="ps2", name="ps2")
                nc.tensor.matmul(ps2, lhsT=w2_sb, rhs=y1, start=True, stop=True)
                y2 = mids.tile([H2, CH], F16, tag="y2", name="y2", bufs=4)
                nc.scalar.activation(y2, ps2, AF.Sign,
                                     bias=cs2_sb[:, 1:2], scale=cs2_sb[:, 0:1])
                return y2

            outts = {}

            def stage_b(c, y2):
                """fc5 -> log_softmax for one chunk; writes half c%2 of the
                pair's [128, 2*4*O] output tile."""
                half = c % 2
                if half == 0:
                    outts[c // 2] = outs.tile([128, 8 * O], F32, tag="out",
                                              name="out_t")
                out_t = outts[c // 2]
                ps5 = p5pool.tile([128, 4, O], F32, tag="ps5", name="ps5")
                for j in range(4):
                    nc.tensor.matmul(ps5[:, j, :], lhsT=y2[:, j * 128:(j + 1) * 128],
                                     rhs=w5_sb, start=True, stop=True)

                # log_softmax along free dim; b5 folded in via exp(b5) weights
                # (|logits| <= 21 so no max-subtraction is needed)
                e = mids.tile([128, 4, O], F32, tag="e", name="e")
                nc.scalar.activation(e, ps5, AF.Exp)
                e2 = mids.tile([128, 4, O], F32, tag="e2", name="e2")
                nc.vector.tensor_tensor(
                    out=e2, in0=e, in1=eb5r_sb.rearrange("p (j o) -> p j o", o=O),
                    op=OP.mult)
                s = mids.tile([128, 4], F32, tag="s", name="s")
                nc.vector.reduce_sum(s, e2, axis=AX.X)
                lse = mids.tile([128, 4], F32, tag="lse", name="lse")
                nc.scalar.activation(lse, s, AF.Ln)
                for j in range(4):
                    js = slice(j * O, (j + 1) * O)
                    nc.vector.scalar_tensor_tensor(
                        out=out_t[:, half * 4 * O + j * O:half * 4 * O + (j + 1) * O],
                        in0=ps5[:, j, :], scalar=lse[:, j:j + 1],
                        in1=b5r_sb[:, js], op0=OP.subtract, op1=OP.add)

            def store(pr, last=False):
                # one [128, 80] transpose covers both chunks of the pair
                psT = pTpool.tile([8 * O, 128], F32, tag="psT", name="psT")
                nc.tensor.transpose(psT, outts.pop(pr), ident)
                oT = outs.tile([8 * O, 128], F32, tag="oT", name="oT")
                nc.vector.tensor_copy(oT, psT)
                # scalar (ACT) HWDGE ring: otherwise unused, and HWDGE has
                # lower completion latency than SWDGE for the final store.
                nc.scalar.dma_start(out=y[:, pr * 128:(pr + 1) * 128], in_=oT)

            # Software pipeline, one chunk per iteration:
            #   fc1(c) | stage_a(c-1) | stage_b(c-2) | store pair (c-3)//2 |
            #   fillers. Every PE instruction's inputs are >=1 iteration old
            #   when the PE's in-order queue reaches it, so the PE only ever
            #   waits for the x DMA -- and the fillers bridge that gap.
            fillers(WARMUP_FILL)
            ps1s = {}
            y2s = {}
            for c in range(nch):
                ps1s[c] = fc1(c)
                if c >= 1:
                    y2s[c - 1] = stage_a(ps1s.pop(c - 1))
                if c >= 2:
                    stage_b(c - 2, y2s.pop(c - 2))
                if c >= 3 and (c - 3) % 2 == 0:
                    store((c - 3) // 2)
                if c + NBUF < nch:
                    issue_x(c + NBUF)
                nf = FILL_AT.get(c, FILL_DEFAULT)
                if nf:
                    fillers(nf, xt=xts[c][0])

            # drain
            y2s[nch - 1] = stage_a(ps1s.pop(nch - 1))
            stage_b(nch - 2, y2s.pop(nch - 2))
            stage_b(nch - 1, y2s.pop(nch - 1))
            store(npair - 1, last=True)

    nc.finalize()
    return nc


def _prep_inputs(x, w1, b1, g1, be1, m1, v1, w2, b2, g2, be2, m2, v2, w5, b5,
                 bpc: int = BPC, n_cores: int = N_CORES):
    f64 = np.float64
    w1s = np.where(w1 >= 0, 1.0, -1.0).astype(np.float32)
    w2s = np.where(w2 >= 0, 1.0, -1.0).astype(np.float32)
    w5s = np.where(w5 >= 0, 1.0, -1.0).astype(np.float32)

    w1h = np.ascontiguousarray(w1s.T).astype(np.float16)          # [D, H1]
    w1l = (np.ascontiguousarray(w1s.T) / LO).astype(np.float16)   # +-2**-11
    w2t = np.ascontiguousarray(w2s.T).astype(np.float16)          # [H1, H2]
    w5t = np.ascontiguousarray(w5s.T).astype(np.float16)          # [H2, O]

    b5f = b5.astype(np.float32)
    b5r = np.broadcast_to(np.tile(b5f, 4)[None, :], (128, 4 * O)).copy()
    eb5 = np.exp(b5.astype(f64)).astype(np.float32)
    eb5r = np.broadcast_to(np.tile(eb5, 4)[None, :], (128, 4 * O)).copy()

    s1 = (g1.astype(f64) / np.sqrt(v1.astype(f64) + EPS))
    t1 = s1 * (b1.astype(f64) - m1.astype(f64)) + be1.astype(f64)
    cs1 = np.stack([s1, t1], axis=1).astype(np.float32)           # [H1, 2]
    s2 = (g2.astype(f64) / np.sqrt(v2.astype(f64) + EPS))
    t2 = s2 * (b2.astype(f64) - m2.astype(f64)) + be2.astype(f64)
    cs2 = np.stack([s2, t2], axis=1).astype(np.float32)           # [H2, 2]

    x = np.asarray(x, dtype=np.float32)
    xh = x.astype(np.float16)
    xl = ((x - xh.astype(np.float32)) * LO).astype(np.float16)

    def swizzle(a):  # [bpc, D] -> [128, nch, KS, CH] (see build_bass)
        nch = bpc // CH
        return np.ascontiguousarray(
            a.T.reshape(KS, 128, nch, CH).transpose(1, 2, 0, 3))

    in_maps = []
    for c in range(n_cores):
        rs = slice(c * bpc, (c + 1) * bpc)
        x2 = np.ascontiguousarray(
            np.stack([swizzle(xh[rs]), swizzle(xl[rs])], axis=2))
        in_maps.append({
            "x2": x2,
            "w1h": w1h, "w1l": w1l, "w2t": w2t, "w5t": w5t,
            "cs1": cs1, "cs2": cs2, "b5r": b5r, "eb5r": eb5r,
        })
    return in_maps


def _decode_output(y_dev: np.ndarray, bpc: int) -> np.ndarray:
    # y_dev [8*O, npair*128]: y_dev[c*4*O+j*O+o, pr*128+p]
    #   = out[pr*1024 + c*512 + j*128 + p, o]
    npair = bpc // (2 * CH)
    return np.ascontiguousarray(
        y_dev.reshape(2, 4, O, npair, 128).transpose(3, 0, 1, 4, 2).reshape(bpc, O))


_CACHED = {}


def kernel(**inputs) -> np.ndarray:
    from concourse.bass_utils import run_bass_kernel_spmd

    in_maps = _prep_inputs(**inputs)
    if "nc" not in _CACHED:
        _CACHED["nc"] = build_bass()
    nc = _CACHED["nc"]
    res = run_bass_kernel_spmd(nc, in_maps, list(range(N_CORES)))
    out = np.empty((B, O), dtype=np.float32)
    for c in range(N_CORES):
        out[c * BPC:(c + 1) * BPC] = _decode_output(res.results[c]["y"], BPC)
    return out


# revision 17
# speedup vs baseline: 1.1698x; 1.1698x over previous
# Trainium2 Bass kernel for a binarized 2-block MLP (BNN):
#   h1 = sign(BN1(x @ sign(w1).T + b1)); h2 = sign(BN2(h1 @ sign(w2).T + b2))
#   out = log_softmax(h2 @ sign(w5).T + b5)
#
# Strategy: pure data parallel over 8 NeuronCores (batch sharded, weights
# replicated). Host-side prep:
#   * x is split into fp16 hi/lo parts (x == xh + xl/2048 to ~2^-22 in fp32).
#     Both stream through the PE at full (1 col/cycle) rate vs fp32's 4
#     passes; with +-1 binary weights every product is exact in the PE's
#     FP22 pipe, so the result matches a true fp32 matmul to accumulation
#     order. hi/lo are interleaved per chunk in ONE dram tensor so each
#     512-row chunk is a single 2MB DMA with 16KB-contiguous runs per
#     partition (HBM line rate).
#   * BN is folded into per-feature scale/shift applied inside the Sign
#     activation (ACT computes func(scale*in + bias) for free).
#   * b5 is folded in on the vector engine: sum(exp(mm+b5)) via a fused
#     multiply-reduce against exp(b5), and the final subtract adds b5 in the
#     same fused op.
#
# Schedule (v2): chunk-granular (512 rows) software pipeline. The kernel is
# HBM-bound: per core 32MB of x must stream at ~358GB/s (~90us) while the PE
# only has ~70us of work, so the PE necessarily idles ~1.3us per chunk. Left
# alone, those idle gaps re-throttle the PE's HAM clock gate to 1.2GHz
# (observed: 58us of K=4/8 in the v1 trace), making the PE the bottleneck.
# Fixes:
#   * fc5's PSUM (and all inter-stage PSUM) double-buffered -- v1 had
#     bufs=1 on ps5, forcing each chunk's fc5 to wait for the previous
#     chunk's full softmax chain (~2.5us PE stall per pair).
#   * "keep-warm" filler matmuls (fp32 ident @ ident, one long accumulation
#     group into a junk PSUM bank) pad the PE's DMA-wait gaps so the HAM
#     activity monitor never sees an idle window: warmup burst at t=0, then
#     a few per chunk.
#   * x chunks stream on the sync (SP) HWDGE ring with nothing else on it;
#     weights/consts go on the scalar (ACT) HWDGE ring.
#   * output stores go via gpsimd SWDGE (keeps SP free); the final store
#     uses the by-then-idle sync ring for lower latency.
# fc5 output lands batch-on-partitions (y2 block as the stationary operand)
# so log_softmax reduces along the free dim; the result is PE-transposed so
# the store DMA has 512B-contiguous runs; host reassembles.

import os
import sys

import numpy as np

for _p in ("/opt/trn_rl_repo", "/root/.axon_site/_ro/trn_rl_repo"):
    if os.path.isdir(_p) and _p not in sys.path:
        sys.path.insert(0, _p)

import concourse.bass as bass
import concourse.mybir as mybir
import concourse.tile as tile
from concourse import bacc
from concourse.masks import make_identity

N_CORES = 8
B, D, H1, H2, O = 65536, 1024, 50, 20, 10
BPC = B // N_CORES  # batch rows per core
CH = 512            # batch chunk (one PSUM bank of fp32)
KS = D // 128       # contraction slices
EPS = 1e-4
LO = 2048.0         # lo-part scale (2**11)

NBUF = 6            # x chunk buffers in flight (absorbs PE lag jitter so the
                    # DMA stream never stalls on buffer recycling)
WARMUP_FILL = 20    # filler matmuls before chunk 0 lands (~6.5us of PE busy)
# per-iteration filler counts: at the real 2.0GHz P0 clock the PE is
# already near-balanced with the ~5.3-5.7us chunk DMA period, so the
# pipeline needs no steady-state fillers (and anchored fillers can stall
# the in-order PE queue if scheduled before their chunk lands).
FILL_AT = {}
FILL_DEFAULT = 0

F16 = mybir.dt.float16
F32 = mybir.dt.float32
AF = mybir.ActivationFunctionType
AX = mybir.AxisListType
OP = mybir.AluOpType


def build_bass(bpc: int = BPC) -> bass.Bass:
    nch = bpc // CH
    npair = nch // 2
    nc = bacc.Bacc("TRN2", target_bir_lowering=False)

    # All activations used here (Sign, Exp, Ln) live together in the
    # "natural_log_exp_and_others" ACT table set, but the default chooser
    # first-matches Exp->exp_and_others and Ln->natural_log, reloading
    # tables twice per chunk (~2.7us each). Restrict the chooser to the
    # combined set (other entries emptied so indices stay aligned with
    # act_info.json) -> exactly one table load for the whole kernel.
    def _act_table_loads_combined_set_only(self=nc):
        import bass_rust as _br

        from concourse.hw_specs import get_activation_tables

        has_act = any(
            isinstance(i, mybir.InstActivation)
            for blk in self.main_func.blocks
            for i in blk.instructions
        )
        if not has_act:
            return
        tabs = get_activation_tables(self.m.arch)
        tables = [
            (name, fns if name == "natural_log_exp_and_others" else set())
            for name, fns in tabs.items()
        ]
        _br.insert_act_table_loads(self, tables)

    nc.insert_act_table_loads = _act_table_loads_combined_set_only

    # x arrives pre-swizzled with hi/lo interleaved per chunk:
    #   x2[p, c, s, k, n] = (xh if s==0 else xl).T[k*128+p, c*CH+n]
    # so each chunk's per-partition DMA slice is one contiguous 16KB run.
    x2 = nc.declare_dram_parameter("x2", [128, nch, 2, KS, CH], F16, isOutput=False)
    w1h = nc.declare_dram_parameter("w1h", [D, H1], F16, isOutput=False)
    w1l = nc.declare_dram_parameter("w1l", [D, H1], F16, isOutput=False)
    w2t = nc.declare_dram_parameter("w2t", [H1, H2], F16, isOutput=False)
    w5t = nc.declare_dram_parameter("w5t", [H2, O], F16, isOutput=False)
    cs1 = nc.declare_dram_parameter("cs1", [H1, 2], F32, isOutput=False)
    cs2 = nc.declare_dram_parameter("cs2", [H2, 2], F32, isOutput=False)
    b5r = nc.declare_dram_parameter("b5r", [128, 4 * O], F32, isOutput=False)
    eb5r = nc.declare_dram_parameter("eb5r", [128, 4 * O], F32, isOutput=False)
    # Output, transposed pair blocks:
    #   y[c*4*O + j*O + o, pr*128 + p] = out[pr*1024 + c*512 + j*128 + p, o]
    y = nc.declare_dram_parameter("y", [8 * O, npair * 128], F32, isOutput=True)

    with tile.TileContext(nc) as tc:
        from contextlib import ExitStack

        with ExitStack() as ctx:
            singles = ctx.enter_context(tc.tile_pool(name="singles", bufs=1))
            xpool = ctx.enter_context(tc.tile_pool(name="xpool", bufs=NBUF))
            mids = ctx.enter_context(tc.tile_pool(name="mids", bufs=3))
            outs = ctx.enter_context(tc.tile_pool(name="outs", bufs=3))
            p1pool = ctx.enter_context(tc.tile_pool(name="p1", bufs=2, space="PSUM"))
            p2pool = ctx.enter_context(tc.tile_pool(name="p2", bufs=2, space="PSUM"))
            p5pool = ctx.enter_context(tc.tile_pool(name="p5", bufs=2, space="PSUM"))
            pTpool = ctx.enter_context(tc.tile_pool(name="pT", bufs=1, space="PSUM"))
            pFpool = ctx.enter_context(tc.tile_pool(name="pF", bufs=1, space="PSUM"))

            # Identity first: the gpsimd engine builds it in ~1us so the
            # warmup fillers can start right after the preamble.
            ident = singles.tile([128, 128], F32)
            make_identity(nc, ident)
            identb = singles.tile([128, 128], F32)
            make_identity(nc, identb)

            # Each x chunk is TWO 1MB DMAs: hi on the sync (SP) HWDGE ring,
            # lo on the gpsimd SWDGE queue. Two reasons:
            #  * The Tile scheduler emits per-engine order from a timed
            #    simulation whose DMA model underestimates HBM rate; with
            #    one queue it concludes the PE must wait for each chunk and
            #    emits a serialized fc1->sign->fc2 chain. Two parallel
            #    queues double the modeled stream rate so the emitted order
            #    keeps the software pipeline.
            #  * Both queues carry the SAME chunk, so the 16 SDMA engines'
            #    packet round-robin still completes chunks in FIFO order at
            #    the full HBM rate. (Alternating whole chunks across queues
            #    instead halves each chunk's rate and delays its completion
            #    semaphore by a full chunk period.)
            # (The lo half must ride the ACT HWDGE ring, not gpsimd SWDGE --
            # SWDGE-generated transfers measured ~25% slower on the wire,
            # dragging the whole stream to ~280GB/s.)
            xts = []

            def issue_x(c):
                xh_t = xpool.tile([128, KS, CH], F16, tag="xh", name="xh_t")
                nc.sync.dma_start(out=xh_t, in_=x2[:, c, 0])
                xl_t = xpool.tile([128, KS, CH], F16, tag="xl", name="xl_t")
                nc.scalar.dma_start(out=xl_t, in_=x2[:, c, 1])
                xts.append((xh_t, xl_t))

            issue_x(0)
            issue_x(1)
            # --- constants / weights: sync ring, slotted behind the first
            # two hi chunks -- they land ~12us in (fc1(0) needs w1 at ~15us)
            # without delaying the first chunk's arrival. ---
            w1h_sb = singles.tile([128, KS, H1], F16)
            nc.sync.dma_start(out=w1h_sb, in_=w1h.rearrange("(k p) m -> p k m", p=128))
            w1l_sb = singles.tile([128, KS, H1], F16)
            nc.sync.dma_start(out=w1l_sb, in_=w1l.rearrange("(k p) m -> p k m", p=128))
            w2_sb = singles.tile([H1, H2], F16)
            nc.sync.dma_start(out=w2_sb, in_=w2t[:, :])
            w5_sb = singles.tile([H2, O], F16)
            nc.sync.dma_start(out=w5_sb, in_=w5t[:, :])
            cs1_sb = singles.tile([H1, 2], F32)
            nc.sync.dma_start(out=cs1_sb, in_=cs1[:, :])
            cs2_sb = singles.tile([H2, 2], F32)
            nc.sync.dma_start(out=cs2_sb, in_=cs2[:, :])
            b5r_sb = singles.tile([128, 4 * O], F32)
            nc.sync.dma_start(out=b5r_sb, in_=b5r[:, :])
            eb5r_sb = singles.tile([128, 4 * O], F32)
            nc.sync.dma_start(out=eb5r_sb, in_=eb5r[:, :])
            for c in range(2, min(NBUF, nch)):
                issue_x(c)

            def fillers(n, xt=None):
                # Keep-warm matmuls: pad PE idle so the HAM activity monitor
                # never re-throttles the clock. One accumulation group -> no
                # per-instruction WAW semaphore stalls; output never read.
                # Warmup form (xt=None): fp32 ident@ident, dependency-free
                # so it runs during the pre-chunk-0 window. In-loop form:
                # reads chunk c's x tile, anchoring it to iteration c (a
                # dependency-free filler would be hoisted to t=0 by the
                # scheduler, bunching all fillers at the start).
                if n <= 0:
                    return
                fp = pFpool.tile([128, CH], F32, tag="f", name="fill")
                if xt is None:
                    for i in range(n):
                        nc.tensor.matmul(fp[:, 0:128],
                                         lhsT=(ident if i % 2 == 0 else identb),
                                         rhs=ident,
                                         start=(i == 0), stop=(i == n - 1))
                else:
                    for i in range(n):
                        nc.tensor.matmul(fp, lhsT=xt[:, 0, 0:128],
                                         rhs=xt[:, 0, :],
                                         start=(i == 0), stop=(i == n - 1))

            def fc1(c):
                xh_t, xl_t = xts[c]
                ps1 = p1pool.tile([H1, CH], F32, tag="ps1", name="ps1")
                # all hi k-slices first: the lo half rides the busier ACT
                # ring and may land ~1us later; its wait sits later in the
                # PE stream so it overlaps the hi matmuls.
                for k in range(KS):
                    nc.tensor.matmul(ps1, lhsT=w1h_sb[:, k, :], rhs=xh_t[:, k, :],
                                     start=(k == 0), stop=False)
                for k in range(KS):
                    nc.tensor.matmul(ps1, lhsT=w1l_sb[:, k, :], rhs=xl_t[:, k, :],
                                     start=False, stop=(k == KS - 1))
                return ps1

            def stage_a(ps1):
                """sign1 -> fc2 -> sign2 for one chunk; returns y2."""
                y1 = mids.tile([H1, CH], F16, tag="y1", name="y1")
                nc.scalar.activation(y1, ps1, AF.Sign,
                                     bias=cs1_sb[:, 1:2], scale=cs1_sb[:, 0:1])
                ps2 = p2pool.tile([H2, CH], F32, tag="ps2", name="ps2")
                nc.tensor.matmul(ps2, lhsT=w2_sb, rhs=y1, start=True, stop=True)
                y2 = mids.tile([H2, CH], F16, tag="y2", name="y2", bufs=4)
                nc.scalar.activation(y2, ps2, AF.Sign,
                                     bias=cs2_sb[:, 1:2], scale=cs2_sb[:, 0:1])
                return y2

            outts = {}

            def stage_b(c, y2):
                """fc5 -> log_softmax for one chunk; writes half c%2 of the
                pair's [128, 2*4*O] output tile."""
                half = c % 2
                if half == 0:
                    outts[c // 2] = outs.tile([128, 8 * O], F32, tag="out",
                                              name="out_t")
                out_t = outts[c // 2]
                ps5 = p5pool.tile([128, 4, O], F32, tag="ps5", name="ps5")
                for j in range(4):
                    nc.tensor.matmul(ps5[:, j, :], lhsT=y2[:, j * 128:(j + 1) * 128],
                                     rhs=w5_sb, start=True, stop=True)

                # log_softmax along free dim; b5 folded in via exp(b5) weights
                # (|logits| <= 21 so no max-subtraction is needed)
                e = mids.tile([128, 4, O], F32, tag="e", name="e")
                nc.scalar.activation(e, ps5, AF.Exp)
                e2 = mids.tile([128, 4, O], F32, tag="e2", name="e2")
                nc.vector.tensor_tensor(
                    out=e2, in0=e, in1=eb5r_sb.rearrange("p (j o) -> p j o", o=O),
                    op=OP.mult)
                s = mids.tile([128, 4], F32, tag="s", name="s")
                nc.vector.reduce_sum(s, e2, axis=AX.X)
                lse = mids.tile([128, 4], F32, tag="lse", name="lse")
                nc.scalar.activation(lse, s, AF.Ln)
                for j in range(4):
                    js = slice(j * O, (j + 1) * O)
                    nc.vector.scalar_tensor_tensor(
                        out=out_t[:, half * 4 * O + j * O:half * 4 * O + (j + 1) * O],
                        in0=ps5[:, j, :], scalar=lse[:, j:j + 1],
                        in1=b5r_sb[:, js], op0=OP.subtract, op1=OP.add)

            def store(pr, last=False):
                # one [128, 80] transpose covers both chunks of the pair
                psT = pTpool.tile([8 * O, 128], F32, tag="psT", name="psT")
                nc.tensor.transpose(psT, outts.pop(pr), ident)
                oT = outs.tile([8 * O, 128], F32, tag="oT", name="oT")
                nc.vector.tensor_copy(oT, psT)
                # SWDGE keeps both HWDGE rings free for x; the last store
                # rides the by-then-idle sync ring (lower latency).
                eng = nc.sync if last else nc.gpsimd
                eng.dma_start(out=y[:, pr * 128:(pr + 1) * 128], in_=oT)

            # Software pipeline, one chunk per iteration:
            #   fc1(c) | stage_a(c-1) | stage_b(c-2) | store pair (c-3)//2 |
            #   fillers. Every PE instruction's inputs are >=1 iteration old
            #   when the PE's in-order queue reaches it, so the PE only ever
            #   waits for the x DMA -- and the fillers bridge that gap.
            fillers(WARMUP_FILL)
            ps1s = {}
            y2s = {}
            for c in range(nch):
                ps1s[c] = fc1(c)
                if c >= 1:
                    y2s[c - 1] = stage_a(ps1s.pop(c - 1))
                if c >= 2:
                    stage_b(c - 2, y2s.pop(c - 2))
                if c >= 3 and (c - 3) % 2 == 0:
                    store((c - 3) // 2)
                if c + NBUF < nch:
                    issue_x(c + NBUF)
                nf = FILL_AT.get(c, FILL_DEFAULT)
                if nf:
                    fillers(nf, xt=xts[c][0])

            # drain
            y2s[nch - 1] = stage_a(ps1s.pop(nch - 1))
            stage_b(nch - 2, y2s.pop(nch - 2))
            stage_b(nch - 1, y2s.pop(nch - 1))
            store(npair - 1, last=True)

    nc.finalize()
    return nc


def _prep_inputs(x, w1, b1, g1, be1, m1, v1, w2, b2, g2, be2, m2, v2, w5, b5,
                 bpc: int = BPC, n_cores: int = N_CORES):
    f64 = np.float64
    w1s = np.where(w1 >= 0, 1.0, -1.0).astype(np.float32)
    w2s = np.where(w2 >= 0, 1.0, -1.0).astype(np.float32)
    w5s = np.where(w5 >= 0, 1.0, -1.0).astype(np.float32)

    w1h = np.ascontiguousarray(w1s.T).astype(np.float16)          # [D, H1]
    w1l = (np.ascontiguousarray(w1s.T) / LO).astype(np.float16)   # +-2**-11
    w2t = np.ascontiguousarray(w2s.T).astype(np.float16)          # [H1, H2]
    w5t = np.ascontiguousarray(w5s.T).astype(np.float16)          # [H2, O]

    b5f = b5.astype(np.float32)
    b5r = np.broadcast_to(np.tile(b5f, 4)[None, :], (128, 4 * O)).copy()
    eb5 = np.exp(b5.astype(f64)).astype(np.float32)
    eb5r = np.broadcast_to(np.tile(eb5, 4)[None, :], (128, 4 * O)).copy()

    s1 = (g1.astype(f64) / np.sqrt(v1.astype(f64) + EPS))
    t1 = s1 * (b1.astype(f64) - m1.astype(f64)) + be1.astype(f64)
    cs1 = np.stack([s1, t1], axis=1).astype(np.float32)           # [H1, 2]
    s2 = (g2.astype(f64) / np.sqrt(v2.astype(f64) + EPS))
    t2 = s2 * (b2.astype(f64) - m2.astype(f64)) + be2.astype(f64)
    cs2 = np.stack([s2, t2], axis=1).astype(np.float32)           # [H2, 2]

    x = np.asarray(x, dtype=np.float32)
    xh = x.astype(np.float16)
    xl = ((x - xh.astype(np.float32)) * LO).astype(np.float16)

    def swizzle(a):  # [bpc, D] -> [128, nch, KS, CH] (see build_bass)
        nch = bpc // CH
        return np.ascontiguousarray(
            a.T.reshape(KS, 128, nch, CH).transpose(1, 2, 0, 3))

    in_maps = []
    for c in range(n_cores):
        rs = slice(c * bpc, (c + 1) * bpc)
        x2 = np.ascontiguousarray(
            np.stack([swizzle(xh[rs]), swizzle(xl[rs])], axis=2))
        in_maps.append({
            "x2": x2,
            "w1h": w1h, "w1l": w1l, "w2t": w2t, "w5t": w5t,
            "cs1": cs1, "cs2": cs2, "b5r": b5r, "eb5r": eb5r,
        })
    return in_maps


def _decode_output(y_dev: np.ndarray, bpc: int) -> np.ndarray:
    # y_dev [8*O, npair*128]: y_dev[c*4*O+j*O+o, pr*128+p]
    #   = out[pr*1024 + c*512 + j*128 + p, o]
    npair = bpc // (2 * CH)
    return np.ascontiguousarray(
        y_dev.reshape(2, 4, O, npair, 128).transpose(3, 0, 1, 4, 2).reshape(bpc, O))


_CACHED = {}


def kernel(**inputs) -> np.ndarray:
    from concourse.bass_utils import run_bass_kernel_spmd

    in_maps = _prep_inputs(**inputs)
    if "nc" not in _CACHED:
        _CACHED["nc"] = build_bass()
    nc = _CACHED["nc"]
    res = run_bass_kernel_spmd(nc, in_maps, list(range(N_CORES)))
    out = np.empty((B, O), dtype=np.float32)
    for c in range(N_CORES):
        out[c * BPC:(c + 1) * BPC] = _decode_output(res.results[c]["y"], BPC)
    return out


# revision 18
# speedup vs baseline: 1.2505x; 1.0689x over previous
# Trainium2 Bass kernel for a binarized 2-block MLP (BNN):
#   h1 = sign(BN1(x @ sign(w1).T + b1)); h2 = sign(BN2(h1 @ sign(w2).T + b2))
#   out = log_softmax(h2 @ sign(w5).T + b5)
#
# Strategy: pure data parallel over 8 NeuronCores (batch sharded, weights
# replicated). Host-side prep:
#   * x is split into fp16 hi/lo parts (x == xh + xl/2048 to ~2^-22 in fp32).
#     Both stream through the PE at full (1 col/cycle) rate vs fp32's 4
#     passes; with +-1 binary weights every product is exact in the PE's
#     FP22 pipe, so the result matches a true fp32 matmul to accumulation
#     order. hi/lo are interleaved per chunk in ONE dram tensor so each
#     512-row chunk is a single 2MB DMA with 16KB-contiguous runs per
#     partition (HBM line rate).
#   * BN is folded into per-feature scale/shift applied inside the Sign
#     activation (ACT computes func(scale*in + bias) for free).
#   * b5 is folded in on the vector engine: sum(exp(mm+b5)) via a fused
#     multiply-reduce against exp(b5), and the final subtract adds b5 in the
#     same fused op.
#
# Schedule (v2): chunk-granular (512 rows) software pipeline. The kernel is
# HBM-bound: per core 32MB of x must stream at ~358GB/s (~90us) while the PE
# only has ~70us of work, so the PE necessarily idles ~1.3us per chunk. Left
# alone, those idle gaps re-throttle the PE's HAM clock gate to 1.2GHz
# (observed: 58us of K=4/8 in the v1 trace), making the PE the bottleneck.
# Fixes:
#   * fc5's PSUM (and all inter-stage PSUM) double-buffered -- v1 had
#     bufs=1 on ps5, forcing each chunk's fc5 to wait for the previous
#     chunk's full softmax chain (~2.5us PE stall per pair).
#   * "keep-warm" filler matmuls (fp32 ident @ ident, one long accumulation
#     group into a junk PSUM bank) pad the PE's DMA-wait gaps so the HAM
#     activity monitor never sees an idle window: warmup burst at t=0, then
#     a few per chunk.
#   * x chunks stream on the sync (SP) HWDGE ring with nothing else on it;
#     weights/consts go on the scalar (ACT) HWDGE ring.
#   * output stores go via gpsimd SWDGE (keeps SP free); the final store
#     uses the by-then-idle sync ring for lower latency.
# fc5 output lands batch-on-partitions (y2 block as the stationary operand)
# so log_softmax reduces along the free dim; the result is PE-transposed so
# the store DMA has 512B-contiguous runs; host reassembles.

import os
import sys

import numpy as np

for _p in ("/opt/trn_rl_repo", "/root/.axon_site/_ro/trn_rl_repo"):
    if os.path.isdir(_p) and _p not in sys.path:
        sys.path.insert(0, _p)

import concourse.bass as bass
import concourse.mybir as mybir
import concourse.tile as tile
from concourse import bacc
from concourse.masks import make_identity

N_CORES = 8
B, D, H1, H2, O = 65536, 1024, 50, 20, 10
BPC = B // N_CORES  # batch rows per core
CH = 512            # batch chunk (one PSUM bank of fp32)
KS = D // 128       # contraction slices
EPS = 1e-4
LO = 2048.0         # lo-part scale (2**11)

NBUF = 6            # x chunk buffers in flight (absorbs PE lag jitter so the
                    # DMA stream never stalls on buffer recycling)
WARMUP_FILL = 20    # filler matmuls before chunk 0 lands (~6.5us of PE busy)
# per-iteration filler counts: at the real 2.0GHz P0 clock the PE is
# already near-balanced with the ~5.3-5.7us chunk DMA period, so the
# pipeline needs no steady-state fillers (and anchored fillers can stall
# the in-order PE queue if scheduled before their chunk lands).
FILL_AT = {}
FILL_DEFAULT = 0

F16 = mybir.dt.float16
F32 = mybir.dt.float32
AF = mybir.ActivationFunctionType
AX = mybir.AxisListType
OP = mybir.AluOpType


def build_bass(bpc: int = BPC) -> bass.Bass:
    nch = bpc // CH
    npair = nch // 2
    nc = bacc.Bacc("TRN2", target_bir_lowering=False)

    # All activations used here (Sign, Exp, Ln) live together in the
    # "natural_log_exp_and_others" ACT table set, but the default chooser
    # first-matches Exp->exp_and_others and Ln->natural_log, reloading
    # tables twice per chunk (~2.7us each). Restrict the chooser to the
    # combined set (other entries emptied so indices stay aligned with
    # act_info.json) -> exactly one table load for the whole kernel.
    def _act_table_loads_combined_set_only(self=nc):
        import bass_rust as _br

        from concourse.hw_specs import get_activation_tables

        has_act = any(
            isinstance(i, mybir.InstActivation)
            for blk in self.main_func.blocks
            for i in blk.instructions
        )
        if not has_act:
            return
        tabs = get_activation_tables(self.m.arch)
        tables = [
            (name, fns if name == "natural_log_exp_and_others" else set())
            for name, fns in tabs.items()
        ]
        _br.insert_act_table_loads(self, tables)

    nc.insert_act_table_loads = _act_table_loads_combined_set_only

    # x arrives pre-swizzled with hi/lo interleaved per chunk:
    #   x2[p, c, s, k, n] = (xh if s==0 else xl).T[k*128+p, c*CH+n]
    # so each chunk's per-partition DMA slice is one contiguous 16KB run.
    x2 = nc.declare_dram_parameter("x2", [128, nch, 2, KS, CH], F16, isOutput=False)
    w1h = nc.declare_dram_parameter("w1h", [D, H1], F16, isOutput=False)
    w1l = nc.declare_dram_parameter("w1l", [D, H1], F16, isOutput=False)
    w2t = nc.declare_dram_parameter("w2t", [H1, H2], F16, isOutput=False)
    w5t = nc.declare_dram_parameter("w5t", [H2, O], F16, isOutput=False)
    cs1 = nc.declare_dram_parameter("cs1", [H1, 2], F32, isOutput=False)
    cs2 = nc.declare_dram_parameter("cs2", [H2, 2], F32, isOutput=False)
    b5r = nc.declare_dram_parameter("b5r", [128, 4 * O], F32, isOutput=False)
    eb5r = nc.declare_dram_parameter("eb5r", [128, 4 * O], F32, isOutput=False)
    # Output, transposed pair blocks:
    #   y[c*4*O + j*O + o, pr*128 + p] = out[pr*1024 + c*512 + j*128 + p, o]
    y = nc.declare_dram_parameter("y", [8 * O, npair * 128], F32, isOutput=True)

    with tile.TileContext(nc) as tc:
        from contextlib import ExitStack

        with ExitStack() as ctx:
            singles = ctx.enter_context(tc.tile_pool(name="singles", bufs=1))
            xpool = ctx.enter_context(tc.tile_pool(name="xpool", bufs=NBUF))
            mids = ctx.enter_context(tc.tile_pool(name="mids", bufs=3))
            outs = ctx.enter_context(tc.tile_pool(name="outs", bufs=3))
            p1pool = ctx.enter_context(tc.tile_pool(name="p1", bufs=2, space="PSUM"))
            p2pool = ctx.enter_context(tc.tile_pool(name="p2", bufs=2, space="PSUM"))
            p5pool = ctx.enter_context(tc.tile_pool(name="p5", bufs=2, space="PSUM"))
            pTpool = ctx.enter_context(tc.tile_pool(name="pT", bufs=1, space="PSUM"))
            pFpool = ctx.enter_context(tc.tile_pool(name="pF", bufs=1, space="PSUM"))

            # Identity first: the gpsimd engine builds it in ~1us so the
            # warmup fillers can start right after the preamble.
            ident = singles.tile([128, 128], F32)
            make_identity(nc, ident)
            identb = singles.tile([128, 128], F32)
            make_identity(nc, identb)

            # Each x chunk is TWO 1MB DMAs: hi on the sync (SP) HWDGE ring,
            # lo on the gpsimd SWDGE queue. Two reasons:
            #  * The Tile scheduler emits per-engine order from a timed
            #    simulation whose DMA model underestimates HBM rate; with
            #    one queue it concludes the PE must wait for each chunk and
            #    emits a serialized fc1->sign->fc2 chain. Two parallel
            #    queues double the modeled stream rate so the emitted order
            #    keeps the software pipeline.
            #  * Both queues carry the SAME chunk, so the 16 SDMA engines'
            #    packet round-robin still completes chunks in FIFO order at
            #    the full HBM rate. (Alternating whole chunks across queues
            #    instead halves each chunk's rate and delays its completion
            #    semaphore by a full chunk period.)
            # (The lo half must ride the ACT HWDGE ring, not gpsimd SWDGE --
            # SWDGE-generated transfers measured ~25% slower on the wire,
            # dragging the whole stream to ~280GB/s.)
            xts = []

            def issue_x(c):
                xh_t = xpool.tile([128, KS, CH], F16, tag="xh", name="xh_t")
                nc.sync.dma_start(out=xh_t, in_=x2[:, c, 0])
                xl_t = xpool.tile([128, KS, CH], F16, tag="xl", name="xl_t")
                nc.scalar.dma_start(out=xl_t, in_=x2[:, c, 1])
                xts.append((xh_t, xl_t))

            # --- fc1 weights lead the sync ring (200KB, land ~9.5us; fc1(0)
            # cannot start without them). The small consts ride between hi
            # chunks -- needed only from stage_a(0) at ~20us. ---
            w1h_sb = singles.tile([128, KS, H1], F16)
            nc.sync.dma_start(out=w1h_sb, in_=w1h.rearrange("(k p) m -> p k m", p=128))
            w1l_sb = singles.tile([128, KS, H1], F16)
            nc.sync.dma_start(out=w1l_sb, in_=w1l.rearrange("(k p) m -> p k m", p=128))
            issue_x(0)
            w2_sb = singles.tile([H1, H2], F16)
            nc.sync.dma_start(out=w2_sb, in_=w2t[:, :])
            w5_sb = singles.tile([H2, O], F16)
            nc.sync.dma_start(out=w5_sb, in_=w5t[:, :])
            cs1_sb = singles.tile([H1, 2], F32)
            nc.sync.dma_start(out=cs1_sb, in_=cs1[:, :])
            cs2_sb = singles.tile([H2, 2], F32)
            nc.sync.dma_start(out=cs2_sb, in_=cs2[:, :])
            b5r_sb = singles.tile([128, 4 * O], F32)
            nc.sync.dma_start(out=b5r_sb, in_=b5r[:, :])
            eb5r_sb = singles.tile([128, 4 * O], F32)
            nc.sync.dma_start(out=eb5r_sb, in_=eb5r[:, :])
            for c in range(1, min(NBUF, nch)):
                issue_x(c)

            def fillers(n, xt=None):
                # Keep-warm matmuls: pad PE idle so the HAM activity monitor
                # never re-throttles the clock. One accumulation group -> no
                # per-instruction WAW semaphore stalls; output never read.
                # Warmup form (xt=None): fp32 ident@ident, dependency-free
                # so it runs during the pre-chunk-0 window. In-loop form:
                # reads chunk c's x tile, anchoring it to iteration c (a
                # dependency-free filler would be hoisted to t=0 by the
                # scheduler, bunching all fillers at the start).
                if n <= 0:
                    return
                fp = pFpool.tile([128, CH], F32, tag="f", name="fill")
                if xt is None:
                    for i in range(n):
                        nc.tensor.matmul(fp[:, 0:128],
                                         lhsT=(ident if i % 2 == 0 else identb),
                                         rhs=ident,
                                         start=(i == 0), stop=(i == n - 1))
                else:
                    for i in range(n):
                        nc.tensor.matmul(fp, lhsT=xt[:, 0, 0:128],
                                         rhs=xt[:, 0, :],
                                         start=(i == 0), stop=(i == n - 1))

            def fc1(c):
                xh_t, xl_t = xts[c]
                ps1 = p1pool.tile([H1, CH], F32, tag="ps1", name="ps1")
                # all hi k-slices first: the lo half rides the busier ACT
                # ring and may land ~1us later; its wait sits later in the
                # PE stream so it overlaps the hi matmuls.
                for k in range(KS):
                    nc.tensor.matmul(ps1, lhsT=w1h_sb[:, k, :], rhs=xh_t[:, k, :],
                                     start=(k == 0), stop=False)
                for k in range(KS):
                    nc.tensor.matmul(ps1, lhsT=w1l_sb[:, k, :], rhs=xl_t[:, k, :],
                                     start=False, stop=(k == KS - 1))
                return ps1

            def stage_a(ps1):
                """sign1 -> fc2 -> sign2 for one chunk; returns y2."""
                y1 = mids.tile([H1, CH], F16, tag="y1", name="y1")
                nc.scalar.activation(y1, ps1, AF.Sign,
                                     bias=cs1_sb[:, 1:2], scale=cs1_sb[:, 0:1])
                ps2 = p2pool.tile([H2, CH], F32, tag="ps2", name="ps2")
                nc.tensor.matmul(ps2, lhsT=w2_sb, rhs=y1, start=True, stop=True)
                y2 = mids.tile([H2, CH], F16, tag="y2", name="y2", bufs=4)
                nc.scalar.activation(y2, ps2, AF.Sign,
                                     bias=cs2_sb[:, 1:2], scale=cs2_sb[:, 0:1])
                return y2

            outts = {}

            def stage_b(c, y2):
                """fc5 -> log_softmax for one chunk; writes half c%2 of the
                pair's [128, 2*4*O] output tile."""
                half = c % 2
                if half == 0:
                    outts[c // 2] = outs.tile([128, 8 * O], F32, tag="out",
                                              name="out_t")
                out_t = outts[c // 2]
                ps5 = p5pool.tile([128, 4, O], F32, tag="ps5", name="ps5")
                for j in range(4):
                    nc.tensor.matmul(ps5[:, j, :], lhsT=y2[:, j * 128:(j + 1) * 128],
                                     rhs=w5_sb, start=True, stop=True)

                # log_softmax along free dim; b5 folded in via exp(b5) weights
                # (|logits| <= 21 so no max-subtraction is needed)
                e = mids.tile([128, 4, O], F32, tag="e", name="e")
                nc.scalar.activation(e, ps5, AF.Exp)
                e2 = mids.tile([128, 4, O], F32, tag="e2", name="e2")
                nc.vector.tensor_tensor(
                    out=e2, in0=e, in1=eb5r_sb.rearrange("p (j o) -> p j o", o=O),
                    op=OP.mult)
                s = mids.tile([128, 4], F32, tag="s", name="s")
                nc.vector.reduce_sum(s, e2, axis=AX.X)
                lse = mids.tile([128, 4], F32, tag="lse", name="lse")
                nc.scalar.activation(lse, s, AF.Ln)
                for j in range(4):
                    js = slice(j * O, (j + 1) * O)
                    nc.vector.scalar_tensor_tensor(
                        out=out_t[:, half * 4 * O + j * O:half * 4 * O + (j + 1) * O],
                        in0=ps5[:, j, :], scalar=lse[:, j:j + 1],
                        in1=b5r_sb[:, js], op0=OP.subtract, op1=OP.add)

            def store(pr, last=False):
                # one [128, 80] transpose covers both chunks of the pair
                psT = pTpool.tile([8 * O, 128], F32, tag="psT", name="psT")
                nc.tensor.transpose(psT, outts.pop(pr), ident)
                oT = outs.tile([8 * O, 128], F32, tag="oT", name="oT")
                nc.vector.tensor_copy(oT, psT)
                # SWDGE keeps both HWDGE rings free for x; the last store
                # rides the by-then-idle sync ring (lower latency).
                eng = nc.sync if last else nc.gpsimd
                eng.dma_start(out=y[:, pr * 128:(pr + 1) * 128], in_=oT)

            # Software pipeline, one chunk per iteration:
            #   fc1(c) | stage_a(c-1) | stage_b(c-2) | store pair (c-3)//2 |
            #   fillers. Every PE instruction's inputs are >=1 iteration old
            #   when the PE's in-order queue reaches it, so the PE only ever
            #   waits for the x DMA -- and the fillers bridge that gap.
            fillers(WARMUP_FILL)
            ps1s = {}
            y2s = {}
            for c in range(nch):
                ps1s[c] = fc1(c)
                if c >= 1:
                    y2s[c - 1] = stage_a(ps1s.pop(c - 1))
                if c >= 2:
                    stage_b(c - 2, y2s.pop(c - 2))
                if c >= 3 and (c - 3) % 2 == 0:
                    store((c - 3) // 2)
                if c + NBUF < nch:
                    issue_x(c + NBUF)
                nf = FILL_AT.get(c, FILL_DEFAULT)
                if nf:
                    fillers(nf, xt=xts[c][0])

            # drain
            y2s[nch - 1] = stage_a(ps1s.pop(nch - 1))
            stage_b(nch - 2, y2s.pop(nch - 2))
            stage_b(nch - 1, y2s.pop(nch - 1))
            store(npair - 1, last=True)

    nc.finalize()
    return nc


def _prep_inputs(x, w1, b1, g1, be1, m1, v1, w2, b2, g2, be2, m2, v2, w5, b5,
                 bpc: int = BPC, n_cores: int = N_CORES):
    f64 = np.float64
    w1s = np.where(w1 >= 0, 1.0, -1.0).astype(np.float32)
    w2s = np.where(w2 >= 0, 1.0, -1.0).astype(np.float32)
    w5s = np.where(w5 >= 0, 1.0, -1.0).astype(np.float32)

    w1h = np.ascontiguousarray(w1s.T).astype(np.float16)          # [D, H1]
    w1l = (np.ascontiguousarray(w1s.T) / LO).astype(np.float16)   # +-2**-11
    w2t = np.ascontiguousarray(w2s.T).astype(np.float16)          # [H1, H2]
    w5t = np.ascontiguousarray(w5s.T).astype(np.float16)          # [H2, O]

    b5f = b5.astype(np.float32)
    b5r = np.broadcast_to(np.tile(b5f, 4)[None, :], (128, 4 * O)).copy()
    eb5 = np.exp(b5.astype(f64)).astype(np.float32)
    eb5r = np.broadcast_to(np.tile(eb5, 4)[None, :], (128, 4 * O)).copy()

    s1 = (g1.astype(f64) / np.sqrt(v1.astype(f64) + EPS))
    t1 = s1 * (b1.astype(f64) - m1.astype(f64)) + be1.astype(f64)
    cs1 = np.stack([s1, t1], axis=1).astype(np.float32)           # [H1, 2]
    s2 = (g2.astype(f64) / np.sqrt(v2.astype(f64) + EPS))
    t2 = s2 * (b2.astype(f64) - m2.astype(f64)) + be2.astype(f64)
    cs2 = np.stack([s2, t2], axis=1).astype(np.float32)           # [H2, 2]

    x = np.asarray(x, dtype=np.float32)
    xh = x.astype(np.float16)
    xl = ((x - xh.astype(np.float32)) * LO).astype(np.float16)

    def swizzle(a):  # [bpc, D] -> [128, nch, KS, CH] (see build_bass)
        nch = bpc // CH
        return np.ascontiguousarray(
            a.T.reshape(KS, 128, nch, CH).transpose(1, 2, 0, 3))

    in_maps = []
    for c in range(n_cores):
        rs = slice(c * bpc, (c + 1) * bpc)
        x2 = np.ascontiguousarray(
            np.stack([swizzle(xh[rs]), swizzle(xl[rs])], axis=2))
        in_maps.append({
            "x2": x2,
            "w1h": w1h, "w1l": w1l, "w2t": w2t, "w5t": w5t,
            "cs1": cs1, "cs2": cs2, "b5r": b5r, "eb5r": eb5r,
        })
    return in_maps


def _decode_output(y_dev: np.ndarray, bpc: int) -> np.ndarray:
    # y_dev [8*O, npair*128]: y_dev[c*4*O+j*O+o, pr*128+p]
    #   = out[pr*1024 + c*512 + j*128 + p, o]
    npair = bpc // (2 * CH)
    return np.ascontiguousarray(
        y_dev.reshape(2, 4, O, npair, 128).transpose(3, 0, 1, 4, 2).reshape(bpc, O))


_CACHED = {}


def kernel(**inputs) -> np.ndarray:
    from concourse.bass_utils import run_bass_kernel_spmd

    in_maps = _prep_inputs(**inputs)
    if "nc" not in _CACHED:
        _CACHED["nc"] = build_bass()
    nc = _CACHED["nc"]
    res = run_bass_kernel_spmd(nc, in_maps, list(range(N_CORES)))
    out = np.empty((B, O), dtype=np.float32)
    for c in range(N_CORES):
        out[c * BPC:(c + 1) * BPC] = _decode_output(res.results[c]["y"], BPC)
    return out
